# revision 13
# baseline (speedup 1.0000x reference)
"""Trainium2 Bass kernel for nn_DiscreteGNNPolicy (3-layer edge-featured GAT).

Strategy (graph/data parallel over 8 NeuronCores):
  - Nodes are sharded by contiguous range: 6272 virtual nodes per core
    (50176 total, >= 50000 real).  Edges are routed on the host to the core
    owning their dst node, sorted by dst, and packed into 128-edge blocks
    grouped by 32-node windows (host does routing/permutation only).
  - Per layer, each core densely computes a per-node "T row"
    [xs interleaved with ones (132) | a_src (4) | a_dst (4) | pad to 256]
    (fp16) for its node slice; slices are AllGathered into a full T table
    in DRAM.
  - Edge phase: per group (128 nodes = 4 windows), dma_gather fetches
    T[src] rows (512B each; the int16-index limit is handled by routing
    each 128-edge block to be homogeneous in src < 32768 vs >= 32768 and
    gathering the "hi" blocks from an offset view of the table), and a
    second dma_gather fetches the dst rows' a_dst columns from the LOCAL
    slice (dst is always local).  exp(leaky_relu(logit)) is folded into a
    sparse selection matrix A[e,(h,j)] = p[e,h] * [dst_local[e]==j], and
    the segment-softmax numerator+denominator are ONE PE matmul per
    128-edge block accumulating into PSUM:
        out[(h,j), c] += sum_e A[e,(h,j)] * G[e,c]
    (the interleaved ones-columns of G yield the softmax denominators).
  - Self loops (PyG fill_value='mean') are handled densely per node from
    segment statistics (cnt, sum of edge_attr) computed with the same
    indicator matmuls in layer 0.
  - Softmax never subtracts the segment max: logits are O(1) here, and
    exp(a)/sum(exp(a)) is mathematically identical.

kernel(**inputs) takes the FULL inputs and returns
(action_logits [N*8], value [1,1], node_embs [N,128]) like the reference.
"""

import math
import os

import numpy as np

# ---------------------------------------------------------------- config
N_REAL = 50000
E_REAL = 1600000
H = 4
C = 32
HID = 128
L = 3
NEG = 0.2
NCORES = 8
TROW = 256       # fp16 elems per T row (512B); 0:132 xs+ones, 132:136 asrc,
                 # 136:140 adst, rest pad
TUSE = 140
LOSPLIT = 32768  # dma_gather int16 index limit


class Cfg:
    def __init__(self, n_real, npc, nw=32):
        self.NREAL = n_real
        self.NPC = npc                  # nodes per core (multiple of 128)
        self.NV = NCORES * npc          # virtual (padded) node count
        self.NW = nw                    # window = nodes per PSUM row set
        self.WPC = npc // nw            # windows per core
        self.TPC = npc // 128           # 128-node tiles (= groups) per core
        self.GWIN = 128 // nw           # windows per group (4)
        assert npc % 128 == 0 and nw * self.GWIN == 128


CFG_FULL = Cfg(N_REAL, 6272)


def _wrap16(slot_arr):
    """[128, B] per-slot values -> dma_gather wrapped idx layout [128, B*8]
    (data in first 16 partitions; idx i of a call at (i%16, bstart*8+i//16),
    which for 128-aligned blocks reduces to a per-block transform)."""
    p128, B = slot_arr.shape
    assert p128 == 128
    w = slot_arr.reshape(8, 16, B).transpose(1, 2, 0).reshape(16, B * 8)
    return np.ascontiguousarray(np.tile(w, (8, 1)))


# ---------------------------------------------------------------- host prep
def host_prep(cfg, edge_index, edge_attr):
    """Pure routing/permutation/padding. Returns per-core slot arrays and the
    shared (slo, shi) block schedules."""
    src = np.asarray(edge_index[0]).astype(np.int64)
    dst = np.asarray(edge_index[1]).astype(np.int64)
    ea = np.asarray(edge_attr, np.float32)

    order = np.argsort(dst, kind="stable")
    src_s, dst_s, ea_s = src[order], dst[order], ea[order]
    lo_s = src_s < LOSPLIT
    gwin = dst_s // cfg.NW
    nwin_g = cfg.NV // cfg.NW
    # counts per (window, lo/hi)
    cnt_lo = np.bincount(gwin[lo_s], minlength=nwin_g).reshape(NCORES, cfg.WPC)
    cnt_hi = np.bincount(gwin[~lo_s], minlength=nwin_g).reshape(NCORES, cfg.WPC)
    slo = np.maximum(1, (cnt_lo + 127) // 128).max(axis=0).astype(np.int64)
    shi = ((cnt_hi + 127) // 128).max(axis=0).astype(np.int64)
    if cfg.NV <= LOSPLIT:
        assert shi.sum() == 0

    GW = cfg.GWIN
    nbLo_g = slo.reshape(-1, GW).sum(1)
    nbHi_g = shi.reshape(-1, GW).sum(1)
    gbase = np.concatenate([[0], np.cumsum(nbLo_g + nbHi_g)])
    # per-window block bases in the global slot order
    lo_base = np.zeros(cfg.WPC, np.int64)
    hi_base = np.zeros(cfg.WPC, np.int64)
    for g in range(cfg.TPC):
        accl = gbase[g]
        for wi in range(GW):
            lo_base[g * GW + wi] = accl
            accl += slo[g * GW + wi]
        acch = gbase[g] + nbLo_g[g]
        for wi in range(GW):
            hi_base[g * GW + wi] = acch
            acch += shi[g * GW + wi]
    B2 = int(gbase[-1])

    # sort window-local edges: within each window put lo edges first? No —
    # separate placement per lo/hi below.
    starts_all = np.concatenate([[0], np.cumsum(np.bincount(gwin, minlength=nwin_g))])

    cores = []
    for k in range(NCORES):
        srcv = np.zeros((128, B2), np.int16)
        dstv = np.zeros((128, B2), np.int16)
        dloc = np.full((128, B2), cfg.NW, np.float16)
        eav = np.zeros((128, B2, 3), np.float16)
        for w in range(cfg.WPC):
            g = k * cfg.WPC + w
            s, e = starts_all[g], starts_all[g + 1]
            if e == s:
                continue
            wsrc, wdst, wea = src_s[s:e], dst_s[s:e], ea_s[s:e]
            wlo = wsrc < LOSPLIT
            for sel, base, sub in ((wlo, lo_base[w], 0),
                                   (~wlo, hi_base[w], LOSPLIT)):
                c = int(sel.sum())
                if c == 0:
                    continue
                sl = np.arange(c)
                p = sl % 128
                b = base + sl // 128
                srcv[p, b] = (wsrc[sel] - sub).astype(np.int16)
                dstv[p, b] = (wdst[sel] - k * cfg.NPC).astype(np.int16)
                dloc[p, b] = (wdst[sel] - g * cfg.NW).astype(np.float16)
                eav[p, b, 0] = wea[sel, 0]
                eav[p, b, 1] = wea[sel, 1]
                eav[p, b, 2] = 1.0
        cores.append(dict(idxmain=_wrap16(srcv),
                          dloc=dloc, eav=eav.reshape(128, B2 * 3)))
    return cores, slo, shi, B2


# ---------------------------------------------------------------- builder
def build_nc(cfg, slo, shi):
    import concourse.bacc as bacc
    import concourse.tile as tile
    from concourse import bass, mybir
    from concourse.masks import make_identity

    f32 = mybir.dt.float32
    f16 = mybir.dt.float16
    i16 = mybir.dt.int16
    i32 = mybir.dt.int32
    Alu = mybir.AluOpType
    Act = mybir.ActivationFunctionType
    Ax = mybir.AxisListType

    NPC, NV, NW, WPC, TPC, GWIN = (cfg.NPC, cfg.NV, cfg.NW, cfg.WPC,
                                   cfg.TPC, cfg.GWIN)
    GW = GWIN
    nbLo_g = slo.reshape(-1, GW).sum(1)
    nbHi_g = shi.reshape(-1, GW).sum(1)
    gbase = np.concatenate([[0], np.cumsum(nbLo_g + nbHi_g)]).astype(int)
    B2 = int(gbase[-1])

    nc = bacc.Bacc("TRN2", target_bir_lowering=False, debug=False,
                   num_devices=NCORES, num_swdge_queues=4)
    RG = [list(range(NCORES))]

    # ---------- I/O -------------------------------------------------------
    nf_d = nc.declare_dram_parameter("nf", [NPC, 3], f32, isOutput=False)
    nmask_d = nc.declare_dram_parameter("nmask", [128, TPC], f32, isOutput=False)
    idxm_d = nc.declare_dram_parameter("idxmain", [128, B2 * 8], i16, isOutput=False)
    dloc_d = nc.declare_dram_parameter("dloc", [128, B2], f16, isOutput=False)
    eav_d = nc.declare_dram_parameter("eav", [128, B2 * 3], f16, isOutput=False)
    lw3_d = nc.declare_dram_parameter("lw3", [3 * HID, HID], f32, isOutput=False)
    lew3_d = nc.declare_dram_parameter("lew3", [3 * HID, HID], f32, isOutput=False)
    atts_d = nc.declare_dram_parameter("atts", [3 * H, C], f32, isOutput=False)
    attd_d = nc.declare_dram_parameter("attd", [3 * H, C], f32, isOutput=False)
    atte_d = nc.declare_dram_parameter("atte", [3 * H, C], f32, isOutput=False)
    gbias_d = nc.declare_dram_parameter("gbias", [3, HID], f32, isOutput=False)
    npw_d = nc.declare_dram_parameter("npw", [3, HID], f32, isOutput=False)
    npb_d = nc.declare_dram_parameter("npb", [1, HID], f32, isOutput=False)
    epw_d = nc.declare_dram_parameter("epw", [2, HID], f32, isOutput=False)
    epb_d = nc.declare_dram_parameter("epb", [1, HID], f32, isOutput=False)
    ahw1_d = nc.declare_dram_parameter("ahw1", [HID, HID], f32, isOutput=False)
    ahb1_d = nc.declare_dram_parameter("ahb1", [HID, 1], f32, isOutput=False)
    ahw2_d = nc.declare_dram_parameter("ahw2", [HID, 8], f32, isOutput=False)
    ahb2_d = nc.declare_dram_parameter("ahb2", [1, 8], f32, isOutput=False)
    vhw1_d = nc.declare_dram_parameter("vhw1", [HID, HID], f32, isOutput=False)
    vhb1_d = nc.declare_dram_parameter("vhb1", [HID, 1], f32, isOutput=False)
    vhw2_d = nc.declare_dram_parameter("vhw2", [HID, 1], f32, isOutput=False)
    vhb2_d = nc.declare_dram_parameter("vhb2", [1, 1], f32, isOutput=False)

    embs_d = nc.declare_dram_parameter("out_embs", [NPC, HID], f32, isOutput=True)
    logit_d = nc.declare_dram_parameter("out_logits", [NPC, 8], f32, isOutput=True)
    value_d = nc.declare_dram_parameter("out_value", [1, 1], f32, isOutput=True)

    # ---------- internal DRAM --------------------------------------------
    tslice = [nc.dram_tensor(f"tslice{l}", [NPC, TROW], f16) for l in range(L)]
    tfull = [nc.dram_tensor(f"tfull{l}", [NV, TROW], f16, addr_space="Shared")
             for l in range(L)]
    aetab = [nc.dram_tensor(f"aetab{l}", [128, B2 * 4], f16) for l in range(L)]
    bscr = nc.dram_tensor("bscr", [4, TROW], f32)
    adscr = nc.dram_tensor("adscr", [TPC, 512], f16)
    gsum_in = nc.dram_tensor("gsum_in", [HID, 1], f32)
    gsum_out = nc.dram_tensor("gsum_out", [HID, 1], f32, addr_space="Shared")

    with tile.TileContext(nc) as tc:
        with (tc.tile_pool(name="per", bufs=1) as per,
              tc.tile_pool(name="grp", bufs=2) as grp,
              tc.tile_pool(name="small", bufs=3) as sm,
              tc.tile_pool(name="psw", bufs=2, space="PSUM") as psw,
              tc.tile_pool(name="psd", bufs=2, space="PSUM") as psd,
              tc.tile_pool(name="psg", bufs=1, space="PSUM") as psg):

            # ---------------- persistent inputs ------------------------
            dloc = per.tile([128, B2], f16)
            nmask = per.tile([128, TPC], f32)
            idxm = per.tile([128, B2 * 8], i16)
            nc.sync.dma_start(out=dloc[:], in_=dloc_d[:])
            nc.sync.dma_start(out=nmask[:], in_=nmask_d[:])
            nc.sync.dma_start(out=idxm[:], in_=idxm_d[:])

            ident = per.tile([128, 128], f32)
            make_identity(nc, ident[:])
            iot32 = per.tile([128, NW], i32)
            nc.gpsimd.iota(iot32[:], pattern=[[1, NW]], base=0,
                           channel_multiplier=0)
            iota16 = per.tile([128, NW], f16)
            nc.vector.tensor_copy(out=iota16[:], in_=iot32[:])

            lw_sb = [per.tile([128, 128], f32, tag=f"lw{l}", name=f"lw{l}")
                     for l in range(L)]
            lew_sb = [per.tile([128, 128], f32, tag=f"lew{l}", name=f"lew{l}")
                      for l in range(L)]
            for l in range(L):
                nc.sync.dma_start(out=lw_sb[l][:], in_=lw3_d[l * 128:(l + 1) * 128, :])
                nc.sync.dma_start(out=lew_sb[l][:], in_=lew3_d[l * 128:(l + 1) * 128, :])
            attsb = per.tile([128, 3 * H * C], f32, tag="attsb")
            attdb = per.tile([128, 3 * H * C], f32, tag="attdb")
            atteb = per.tile([128, 3 * H * C], f32, tag="atteb")
            for r in range(3 * H):
                for dsttile, srcd in ((attsb, atts_d), (attdb, attd_d), (atteb, atte_d)):
                    nc.sync.dma_start(
                        out=dsttile[:, r * C:(r + 1) * C],
                        in_=srcd[r:r + 1, :].partition_broadcast(128))
            gbb = [per.tile([128, HID], f32, tag=f"gbb{l}", name=f"gbb{l}")
                   for l in range(L)]
            for l in range(L):
                nc.sync.dma_start(out=gbb[l][:],
                                  in_=gbias_d[l:l + 1, :].partition_broadcast(128))
            npw_sb = per.tile([3, HID], f32)
            npb_sb = per.tile([1, HID], f32)
            epw_sb = per.tile([2, HID], f32)
            epb_sb = per.tile([1, HID], f32)
            nc.sync.dma_start(out=npw_sb[:], in_=npw_d[:])
            nc.sync.dma_start(out=npb_sb[:], in_=npb_d[:])
            nc.sync.dma_start(out=epw_sb[:], in_=epw_d[:])
            nc.sync.dma_start(out=epb_sb[:], in_=epb_d[:])
            ahw1 = per.tile([HID, HID], f32)
            ahb1 = per.tile([HID, 1], f32)
            ahw2 = per.tile([HID, 8], f32)
            ahb2b = per.tile([128, 8], f32)
            vhw1 = per.tile([HID, HID], f32)
            vhb1 = per.tile([HID, 1], f32)
            vhw2 = per.tile([HID, 1], f32)
            vhb2 = per.tile([1, 1], f32)
            nc.sync.dma_start(out=ahw1[:], in_=ahw1_d[:])
            nc.sync.dma_start(out=ahb1[:], in_=ahb1_d[:])
            nc.sync.dma_start(out=ahw2[:], in_=ahw2_d[:])
            nc.sync.dma_start(out=ahb2b[:], in_=ahb2_d[:].partition_broadcast(128))
            nc.sync.dma_start(out=vhw1[:], in_=vhw1_d[:])
            nc.sync.dma_start(out=vhb1[:], in_=vhb1_d[:])
            nc.sync.dma_start(out=vhw2[:], in_=vhw2_d[:])
            nc.sync.dma_start(out=vhb2[:], in_=vhb2_d[:])

            # ---------------- fold weights -----------------------------
            we_sb = per.tile([128, L * H], f32)
            wcat = [per.tile([128, TUSE], f32, tag=f"wcat{l}", name=f"wcat{l}")
                    for l in range(L)]
            for l in range(L):
                nc.vector.memset(wcat[l][:], 0.0)
                nc.vector.tensor_copy(
                    out=wcat[l][:, 0:132].rearrange("p (h c) -> p h c", c=33)[:, :, 0:32],
                    in_=lw_sb[l][:].rearrange("p (h c) -> p h c", c=32))
                for h in range(H):
                    r = l * H + h
                    tmphc = sm.tile([128, C], f32, tag="tmphc")
                    nc.vector.tensor_tensor(
                        out=tmphc[:], in0=lw_sb[l][:, h * C:(h + 1) * C],
                        in1=attsb[:, r * C:(r + 1) * C], op=Alu.mult)
                    nc.vector.tensor_reduce(out=wcat[l][:, 132 + h:133 + h],
                                            in_=tmphc[:], axis=Ax.X, op=Alu.add)
                    tmphc = sm.tile([128, C], f32, tag="tmphc")
                    nc.vector.tensor_tensor(
                        out=tmphc[:], in0=lw_sb[l][:, h * C:(h + 1) * C],
                        in1=attdb[:, r * C:(r + 1) * C], op=Alu.mult)
                    nc.vector.tensor_reduce(out=wcat[l][:, 136 + h:137 + h],
                                            in_=tmphc[:], axis=Ax.X, op=Alu.add)
                    tmphc = sm.tile([128, C], f32, tag="tmphc")
                    nc.vector.tensor_tensor(
                        out=tmphc[:], in0=lew_sb[l][:, h * C:(h + 1) * C],
                        in1=atteb[:, r * C:(r + 1) * C], op=Alu.mult)
                    nc.vector.tensor_reduce(out=we_sb[:, r:r + 1],
                                            in_=tmphc[:], axis=Ax.X, op=Alu.add)

            tp_ps = psd.tile([128, 2], f32, tag="mm")
            nc.tensor.transpose(out=tp_ps[:], in_=epw_sb[:], identity=ident[0:2, 0:2])
            epwT = per.tile([128, 2], f32)
            nc.vector.tensor_copy(out=epwT[:], in_=tp_ps[:])
            tp_ps1 = psd.tile([128, 1], f32, tag="mm")
            nc.tensor.transpose(out=tp_ps1[:], in_=epb_sb[:], identity=ident[0:1, 0:1])
            epbT = per.tile([128, 1], f32)
            nc.vector.tensor_copy(out=epbT[:], in_=tp_ps1[:])
            m2sb = sm.tile([2, L * H], f32, tag="m2sb")
            clsb = sm.tile([1, L * H], f32, tag="clsb")
            for l in range(L):
                mps = psd.tile([2, H], f32, tag="mm")
                nc.tensor.matmul(out=mps[:], lhsT=epwT[:],
                                 rhs=we_sb[:, l * H:(l + 1) * H], start=True, stop=True)
                nc.vector.tensor_copy(out=m2sb[:, l * H:(l + 1) * H], in_=mps[:])
                cps = psd.tile([1, H], f32, tag="mm")
                nc.tensor.matmul(out=cps[:], lhsT=epbT[:],
                                 rhs=we_sb[:, l * H:(l + 1) * H], start=True, stop=True)
                nc.vector.tensor_copy(out=clsb[:, l * H:(l + 1) * H], in_=cps[:])
            nc.sync.dma_start(out=bscr[0:1, 0:L * H], in_=m2sb[0:1, :])
            nc.sync.dma_start(out=bscr[1:2, 0:L * H], in_=m2sb[1:2, :])
            nc.sync.dma_start(out=bscr[2:3, 0:L * H], in_=clsb[:])

            tp_ps3 = psd.tile([128, 3], f32, tag="mm")
            nc.tensor.transpose(out=tp_ps3[:], in_=npw_sb[:],
                                identity=ident[0:3, 0:3])
            npwT = per.tile([128, 3], f32)
            nc.vector.tensor_copy(out=npwT[:], in_=tp_ps3[:])
            m0ps = psd.tile([3, TUSE], f32, tag="mm")
            nc.tensor.matmul(out=m0ps[:], lhsT=npwT[:], rhs=wcat[0][:],
                             start=True, stop=True)
            m0sb = per.tile([3, TUSE], f32)
            nc.vector.tensor_copy(out=m0sb[:], in_=m0ps[:])
            tp_ps1b = psd.tile([128, 1], f32, tag="mm")
            nc.tensor.transpose(out=tp_ps1b[:], in_=npb_sb[:],
                                identity=ident[0:1, 0:1])
            npbT = per.tile([128, 1], f32)
            nc.vector.tensor_copy(out=npbT[:], in_=tp_ps1b[:])
            b0ps = psd.tile([1, TUSE], f32, tag="mm")
            nc.tensor.matmul(out=b0ps[:], lhsT=npbT[:], rhs=wcat[0][:],
                             start=True, stop=True)
            b0sb = sm.tile([1, TUSE], f32, tag="b0sb")
            nc.vector.tensor_copy(out=b0sb[:], in_=b0ps[:])
            nc.sync.dma_start(out=bscr[3:4, 0:TUSE], in_=b0sb[:])

            m2b0 = per.tile([128, L * H], f32)
            m2b1 = per.tile([128, L * H], f32)
            cb12 = per.tile([128, L * H], f32)
            bias0b = per.tile([128, TUSE], f32)
            nc.sync.dma_start(out=m2b0[:], in_=bscr[0:1, 0:L * H].partition_broadcast(128))
            nc.sync.dma_start(out=m2b1[:], in_=bscr[1:2, 0:L * H].partition_broadcast(128))
            nc.sync.dma_start(out=cb12[:], in_=bscr[2:3, 0:L * H].partition_broadcast(128))
            nc.sync.dma_start(out=bias0b[:], in_=bscr[3:4, 0:TUSE].partition_broadcast(128))

            # ---------------- aetab ------------------------------------
            CHK = 512
            NCHUNK = max(1, math.ceil(B2 / CHK))
            for l in range(L):
                for ci in range(NCHUNK):
                    c0 = ci * CHK
                    c1 = min(B2, c0 + CHK)
                    cB = c1 - c0
                    if cB <= 0:
                        continue
                    eavc = sm.tile([128, CHK * 3], f16, tag="eavc", bufs=1)
                    nc.sync.dma_start(out=eavc[:, :cB * 3],
                                      in_=eav_d[:, c0 * 3:c1 * 3])
                    ea3 = eavc[:, :cB * 3].rearrange("p (b e) -> p b e", e=3)
                    t1 = sm.tile([128, CHK * 4], f32, tag="aet1", bufs=1)
                    t2 = sm.tile([128, CHK * 4], f32, tag="aet2", bufs=1)
                    nc.vector.tensor_tensor(
                        out=t1[:, :cB * 4].rearrange("p (b h) -> p b h", h=4),
                        in0=ea3[:, :, 0:1].to_broadcast([128, cB, 4]),
                        in1=m2b0[:, l * 4:(l + 1) * 4].unsqueeze(1).to_broadcast([128, cB, 4]),
                        op=Alu.mult)
                    nc.vector.tensor_tensor(
                        out=t2[:, :cB * 4].rearrange("p (b h) -> p b h", h=4),
                        in0=ea3[:, :, 1:2].to_broadcast([128, cB, 4]),
                        in1=m2b1[:, l * 4:(l + 1) * 4].unsqueeze(1).to_broadcast([128, cB, 4]),
                        op=Alu.mult)
                    nc.vector.tensor_tensor(out=t1[:, :cB * 4], in0=t1[:, :cB * 4],
                                            in1=t2[:, :cB * 4], op=Alu.add)
                    ae16 = sm.tile([128, CHK * 4], f16, tag="ae16", bufs=1)
                    nc.vector.tensor_tensor(
                        out=ae16[:, :cB * 4].rearrange("p (b h) -> p b h", h=4),
                        in0=t1[:, :cB * 4].rearrange("p (b h) -> p b h", h=4),
                        in1=cb12[:, l * 4:(l + 1) * 4].unsqueeze(1).to_broadcast([128, cB, 4]),
                        op=Alu.add)
                    nc.sync.dma_start(out=aetab[l][:, c0 * 4:c1 * 4],
                                      in_=ae16[:, :cB * 4])

            # ---------------- persistent state -------------------------
            tloc = per.tile([128, TPC * TUSE], f16)
            xcur = per.tile([128, TPC * 128], f32)
            la_sb = per.tile([128, TPC * L * H], f32)

            # ---------------- dense phase ------------------------------
            def dense_phase(l):
                for t in range(TPC):
                    TL = tloc[:, t * TUSE:(t + 1) * TUSE]
                    if l == 0:
                        nfT_ps = psd.tile([3, 128], f32, tag="mm")
                        nc.tensor.transpose(out=nfT_ps[:],
                                            in_=nfsb[:, t * 3:(t + 1) * 3],
                                            identity=ident[:])
                        nfT = sm.tile([3, 128], f32, tag="nfT")
                        nc.vector.tensor_copy(out=nfT[:], in_=nfT_ps[:])
                        dps = psd.tile([128, TUSE], f32, tag="mm")
                        nc.tensor.matmul(out=dps[:], lhsT=nfT[:], rhs=m0sb[:],
                                         start=True, stop=True)
                        nc.vector.tensor_tensor(out=TL, in0=dps[:], in1=bias0b[:],
                                                op=Alu.add)
                    else:
                        xT_ps = psd.tile([128, 128], f32, tag="mm")
                        nc.tensor.transpose(out=xT_ps[:],
                                            in_=xcur[:, t * 128:(t + 1) * 128],
                                            identity=ident[:])
                        xT = sm.tile([128, 128], f32, tag="xT")
                        nc.vector.tensor_copy(out=xT[:], in_=xT_ps[:])
                        dps = psd.tile([128, TUSE], f32, tag="mm")
                        nc.tensor.matmul(out=dps[:], lhsT=xT[:], rhs=wcat[l][:],
                                         start=True, stop=True)
                        nc.vector.tensor_copy(out=TL, in_=dps[:])
                    nc.vector.memset(
                        TL[:, 0:132].rearrange("p (h c) -> p h c", c=33)[:, :, 32], 1.0)
                nc.sync.dma_start(
                    out=tslice[l][:].rearrange("(t p) f -> p t f", p=128)[:, :, 0:TUSE],
                    in_=tloc[:].rearrange("p (t f) -> p t f", f=TUSE))
                nc.gpsimd.collective_compute(
                    "AllGather", Alu.bypass, replica_groups=RG,
                    ins=[tslice[l][:]], outs=[tfull[l][:]])

            nfsb = per.tile([128, TPC * 3], f32)
            nc.sync.dma_start(out=nfsb[:],
                              in_=nf_d[:].rearrange("(t p) f -> p t f", p=128))

            # zero the pad columns of every tslice once (the gathers read
            # full 512B rows; uninitialized DRAM would be NaN in sim)
            zpad = per.tile([128, TROW - TUSE], f16)
            nc.vector.memset(zpad[:], 0.0)
            for l in range(L):
                nc.sync.dma_start(
                    out=tslice[l][:].rearrange("(t p) f -> p t f", p=128)[:, :, TUSE:TROW],
                    in_=zpad[:].unsqueeze(1).to_broadcast([128, TPC, TROW - TUSE]))

            dense_phase(0)

            # ---------------- edge phase -------------------------------
            boff_lo = []
            boff_hi = []
            for g in range(TPC):
                accl = 0
                acch = int(nbLo_g[g])
                ls, hs = [], []
                for wi in range(GW):
                    ls.append(accl)
                    hs.append(acch)
                    accl += int(slo[g * GW + wi])
                    acch += int(shi[g * GW + wi])
                boff_lo.append(ls)
                boff_hi.append(hs)
            qrr = [0]
            for l in range(L):
                for g in range(TPC):
                    w0 = g * GW
                    g0 = int(gbase[g])
                    nbLo = int(nbLo_g[g])
                    nbHi = int(nbHi_g[g])
                    nbG = nbLo + nbHi
                    # ---- gathers (<=1024 idx per call, 4 SWDGE queues) ----
                    Gt = grp.tile([128, nbG * TROW], f16, tag="Gt")
                    for c0 in range(0, nbLo, 8):
                        nb = min(8, nbLo - c0)
                        nc.gpsimd.dma_gather(
                            out_ap=Gt[:, (c0) * TROW:(c0 + nb) * TROW]
                                .rearrange("p (b f) -> p b f", f=TROW),
                            in_ap=tfull[l][:, :],
                            idxs_ap=idxm[:, (g0 + c0) * 8:(g0 + c0 + nb) * 8],
                            num_idxs=nb * 128, num_idxs_reg=nb * 128,
                            elem_size=TROW, queue_num=qrr[0] % 4)
                        qrr[0] += 1
                    for c0 in range(0, nbHi, 8):
                        nb = min(8, nbHi - c0)
                        nc.gpsimd.dma_gather(
                            out_ap=Gt[:, (nbLo + c0) * TROW:(nbLo + c0 + nb) * TROW]
                                .rearrange("p (b f) -> p b f", f=TROW),
                            in_ap=tfull[l][LOSPLIT:NV, :],
                            idxs_ap=idxm[:, (g0 + nbLo + c0) * 8:(g0 + nbLo + c0 + nb) * 8],
                            num_idxs=nb * 128, num_idxs_reg=nb * 128,
                            elem_size=TROW, queue_num=qrr[0] % 4)
                        qrr[0] += 1
                    aet = grp.tile([128, nbG * 4], f16, tag="aet")
                    nc.sync.dma_start(out=aet[:],
                                      in_=aetab[l][:, g0 * 4:(g0 + nbG) * 4])
                    # ---- a_dst row broadcast (local tile -> DRAM -> bcast) ----
                    TLg = tloc[:, g * TUSE:(g + 1) * TUSE]
                    nc.sync.dma_start(
                        out=adscr[g, :].rearrange("(p h) -> p h", h=4),
                        in_=TLg[:, 136:140])
                    adb = grp.tile([128, 512], f16, tag="adb")
                    nc.sync.dma_start(out=adb[:],
                                      in_=adscr[g:g + 1, :].partition_broadcast(128))
                    # ---- indicator (needed for a_dst select and A) ----
                    Ig = grp.tile([128, nbG * NW], f16, tag="Ig")
                    nc.vector.tensor_tensor(
                        out=Ig[:].rearrange("p (b j) -> p b j", j=NW),
                        in0=iota16[:].unsqueeze(1).to_broadcast([128, nbG, NW]),
                        in1=dloc[:, g0:g0 + nbG].unsqueeze(2).to_broadcast([128, nbG, NW]),
                        op=Alu.is_equal)
                    # ---- per-edge logits ----
                    sg = grp.tile([128, nbG * 4], f32, tag="sg")
                    G3 = Gt[:].rearrange("p (b f) -> p b f", f=TROW)
                    nc.vector.tensor_tensor(
                        out=sg[:].rearrange("p (b h) -> p b h", h=4),
                        in0=G3[:, :, 132:136],
                        in1=aet[:].rearrange("p (b h) -> p b h", h=4), op=Alu.add)
                    # a_dst select: t = I (*) adst_row ; reduce over j
                    boff = 0
                    for wi in range(GW):
                        nbW = int(slo[w0 + wi]) + int(shi[w0 + wi])
                        blks = ([boff_lo[g][wi] + j for j in range(int(slo[w0 + wi]))] +
                                [boff_hi[g][wi] + j for j in range(int(shi[w0 + wi]))])
                        tsel = grp.tile([128, 8 * 4 * NW], f16, tag="tsel", bufs=3)
                        adp = grp.tile([128, 8 * 4], f32, tag="adp", bufs=3)
                        for ci, gb in enumerate(blks):
                            pass
                        # build t for this window's blocks (they are two
                        # contiguous runs; do each run in one op)
                        runs = [(boff_lo[g][wi], int(slo[w0 + wi])),
                                (boff_hi[g][wi], int(shi[w0 + wi]))]
                        for rb, rn in runs:
                            if rn == 0:
                                continue
                            nc.vector.tensor_tensor(
                                out=tsel[:, :rn * 4 * NW].rearrange(
                                    "p (b h j) -> p b h j", h=4, j=NW),
                                in0=Ig[:, rb * NW:(rb + rn) * NW]
                                    .rearrange("p (b j) -> p b j", j=NW)
                                    .unsqueeze(2).to_broadcast([128, rn, 4, NW]),
                                in1=adb[:, wi * 128:(wi + 1) * 128]
                                    .rearrange("p (j h) -> p j h", h=4)
                                    .transpose([0, 2, 1]).unsqueeze(1)
                                    .to_broadcast([128, rn, 4, NW]),
                                op=Alu.mult)
                            nc.vector.tensor_reduce(
                                out=adp[:, :rn * 4].rearrange("p (b h) -> p b h", h=4),
                                in_=tsel[:, :rn * 4 * NW].rearrange(
                                    "p (b h j) -> p b h j", h=4, j=NW),
                                axis=Ax.X, op=Alu.add)
                            nc.vector.tensor_tensor(
                                out=sg[:, rb * 4:(rb + rn) * 4],
                                in0=sg[:, rb * 4:(rb + rn) * 4],
                                in1=adp[:, :rn * 4], op=Alu.add)
                    lk = grp.tile([128, nbG * 4], f32, tag="lk")
                    lk = grp.tile([128, nbG * 4], f32, tag="lk")
                    nc.vector.tensor_scalar_mul(lk[:], sg[:], NEG)
                    nc.vector.tensor_tensor(out=sg[:], in0=sg[:], in1=lk[:],
                                            op=Alu.max)
                    pg = grp.tile([128, nbG * 4], f16, tag="pg")
                    nc.scalar.activation(out=pg[:], in_=sg[:], func=Act.Exp)
                    # ---- indicator + A ----
                    Ig = grp.tile([128, nbG * NW], f16, tag="Ig")
                    nc.vector.tensor_tensor(
                        out=Ig[:].rearrange("p (b j) -> p b j", j=NW),
                        in0=iota16[:].unsqueeze(1).to_broadcast([128, nbG, NW]),
                        in1=dloc[:, g0:g0 + nbG].unsqueeze(2).to_broadcast([128, nbG, NW]),
                        op=Alu.is_equal)
                    Ag = grp.tile([128, nbG * 4 * NW], f16, tag="Ag")
                    nc.vector.tensor_tensor(
                        out=Ag[:].rearrange("p (b h j) -> p b h j", h=4, j=NW),
                        in0=Ig[:].rearrange("p (b j) -> p b j", j=NW)
                            .unsqueeze(2).to_broadcast([128, nbG, 4, NW]),
                        in1=pg[:].rearrange("p (b h) -> p b h", h=4)
                            .unsqueeze(3).to_broadcast([128, nbG, 4, NW]),
                        op=Alu.mult)

                    if l == 0:
                        eavg = grp.tile([128, nbG * 3], f16, tag="eavg")
                        nc.sync.dma_start(out=eavg[:],
                                          in_=eav_d[:, g0 * 3:(g0 + nbG) * 3])
                        stt = grp.tile([128, 3], f32, tag="stt")
                    agg = grp.tile([128, 132], f32, tag="agg")
                    for wi in range(GW):
                        w = w0 + wi
                        blocks = ([boff_lo[g][wi] + j for j in range(int(slo[w]))] +
                                  [boff_hi[g][wi] + j for j in range(int(shi[w]))])
                        pw = psw.tile([4 * NW, 136], f32, tag="pw")
                        for bi, gb in enumerate(blocks):
                            nc.tensor.matmul(
                                out=pw[:], lhsT=Ag[:, gb * 128:(gb + 1) * 128],
                                rhs=Gt[:, gb * TROW:gb * TROW + 136],
                                start=(bi == 0), stop=(bi == len(blocks) - 1))
                        for h in range(H):
                            dst_ap = agg[wi * NW:(wi + 1) * NW, h * 33:h * 33 + 33]
                            src_ap = pw[h * NW:(h + 1) * NW, h * 33:h * 33 + 33]
                            if h % 2 == 0:
                                nc.scalar.copy(out=dst_ap, in_=src_ap)
                            else:
                                nc.vector.tensor_copy(out=dst_ap, in_=src_ap)
                        if l == 0:
                            pst = psw.tile([NW, 3], f32, tag="pst", bufs=1)
                            for bi, gb in enumerate(blocks):
                                nc.tensor.matmul(
                                    out=pst[:], lhsT=Ig[:, gb * NW:(gb + 1) * NW],
                                    rhs=eavg[:, gb * 3:gb * 3 + 3],
                                    start=(bi == 0), stop=(bi == len(blocks) - 1))
                            nc.vector.tensor_copy(
                                out=stt[wi * NW:(wi + 1) * NW, :], in_=pst[:])

                    # ---- group epilogue ----
                    TL = tloc[:, g * TUSE:(g + 1) * TUSE]
                    if l == 0:
                        cmax = sm.tile([128, 1], f32, tag="cmax")
                        nc.vector.tensor_scalar_max(cmax[:], stt[:, 2:3], 1.0)
                        rec = sm.tile([128, 1], f32, tag="rec")
                        nc.vector.reciprocal(out=rec[:], in_=cmax[:])
                        mea = sm.tile([128, 2], f32, tag="mea")
                        nc.vector.tensor_tensor(out=mea[:], in0=stt[:, 0:2],
                                                in1=rec[:].to_broadcast([128, 2]),
                                                op=Alu.mult)
                        lt1 = sm.tile([128, L * H], f32, tag="lt1")
                        lt2 = sm.tile([128, L * H], f32, tag="lt2")
                        nc.vector.tensor_tensor(
                            out=lt1[:], in0=mea[:, 0:1].to_broadcast([128, L * H]),
                            in1=m2b0[:], op=Alu.mult)
                        nc.vector.tensor_tensor(
                            out=lt2[:], in0=mea[:, 1:2].to_broadcast([128, L * H]),
                            in1=m2b1[:], op=Alu.mult)
                        nc.vector.tensor_tensor(out=lt1[:], in0=lt1[:], in1=lt2[:],
                                                op=Alu.add)
                        nc.vector.tensor_tensor(
                            out=la_sb[:, g * L * H:(g + 1) * L * H],
                            in0=lt1[:], in1=cb12[:], op=Alu.add)
                    slp = sm.tile([128, 4], f32, tag="slp")
                    nc.vector.tensor_tensor(out=slp[:], in0=TL[:, 132:136],
                                            in1=TL[:, 136:140], op=Alu.add)
                    nc.vector.tensor_tensor(
                        out=slp[:], in0=slp[:],
                        in1=la_sb[:, g * L * H + l * 4:g * L * H + l * 4 + 4],
                        op=Alu.add)
                    slk = sm.tile([128, 4], f32, tag="slk")
                    nc.vector.tensor_scalar_mul(slk[:], slp[:], NEG)
                    nc.vector.tensor_tensor(out=slp[:], in0=slp[:], in1=slk[:],
                                            op=Alu.max)
                    pl = sm.tile([128, 4], f32, tag="pl")
                    nc.scalar.activation(out=pl[:], in_=slp[:], func=Act.Exp)
                    tmp132 = sm.tile([128, 132], f32, tag="tmp132")
                    nc.vector.tensor_tensor(
                        out=tmp132[:].rearrange("p (h c) -> p h c", c=33),
                        in0=TL[:, 0:132].rearrange("p (h c) -> p h c", c=33),
                        in1=pl[:].unsqueeze(2).to_broadcast([128, 4, 33]),
                        op=Alu.mult)
                    nc.vector.tensor_tensor(out=agg[:], in0=agg[:], in1=tmp132[:],
                                            op=Alu.add)
                    rec4 = sm.tile([128, 4], f32, tag="rec4")
                    nc.vector.reciprocal(
                        out=rec4[:],
                        in_=agg[:, 0:132].rearrange("p (h c) -> p h c", c=33)[:, :, 32])
                    xt = sm.tile([128, 128], f32, tag="xtm")
                    nc.vector.tensor_tensor(
                        out=xt[:].rearrange("p (h c) -> p h c", c=32),
                        in0=agg[:, 0:132].rearrange("p (h c) -> p h c", c=33)[:, :, 0:32],
                        in1=rec4[:].unsqueeze(2).to_broadcast([128, 4, 32]),
                        op=Alu.mult)
                    nc.vector.tensor_tensor(out=xt[:], in0=xt[:], in1=gbb[l][:],
                                            op=Alu.add)
                    xm = sm.tile([128, 128], f32, tag="xm")
                    nc.vector.tensor_scalar_min(xm[:], xt[:], 0.0)
                    xe = sm.tile([128, 128], f32, tag="xe")
                    nc.scalar.activation(out=xe[:], in_=xm[:], func=Act.Exp)
                    xr = sm.tile([128, 128], f32, tag="xr")
                    nc.vector.tensor_scalar_max(xr[:], xt[:], 0.0)
                    nc.vector.tensor_tensor(out=xr[:], in0=xr[:], in1=xe[:],
                                            op=Alu.add)
                    nc.vector.tensor_scalar_add(
                        xcur[:, g * 128:(g + 1) * 128], xr[:], -1.0)

                if l + 1 < L:
                    dense_phase(l + 1)

            # ---------------- heads -----------------------------------
            lg_sb = per.tile([128, TPC * 8], f32)
            gps = psg.tile([128, 1], f32, tag="gps")
            for t in range(TPC):
                xT_ps = psd.tile([128, 128], f32, tag="mm")
                nc.tensor.transpose(out=xT_ps[:], in_=xcur[:, t * 128:(t + 1) * 128],
                                    identity=ident[:])
                xT = sm.tile([128, 128], f32, tag="xT")
                nc.vector.tensor_copy(out=xT[:], in_=xT_ps[:])
                h1ps = psd.tile([128, 128], f32, tag="mm")
                nc.tensor.matmul(out=h1ps[:], lhsT=ahw1[:], rhs=xT[:],
                                 start=True, stop=True)
                h1T = sm.tile([128, 128], f32, tag="h1T")
                nc.scalar.activation(out=h1T[:], in_=h1ps[:], func=Act.Relu,
                                     bias=ahb1[:, 0:1])
                lgps = psd.tile([128, 8], f32, tag="mm")
                nc.tensor.matmul(out=lgps[:], lhsT=h1T[:], rhs=ahw2[:],
                                 start=True, stop=True)
                nc.vector.tensor_tensor(out=lg_sb[:, t * 8:(t + 1) * 8],
                                        in0=lgps[:], in1=ahb2b[:], op=Alu.add)
                nc.tensor.matmul(out=gps[:], lhsT=xcur[:, t * 128:(t + 1) * 128],
                                 rhs=nmask[:, t:t + 1],
                                 start=(t == 0), stop=(t == TPC - 1))
            nc.sync.dma_start(
                out=logit_d[:].rearrange("(t p) f -> p t f", p=128),
                in_=lg_sb[:].rearrange("p (t f) -> p t f", f=8))
            nc.sync.dma_start(
                out=embs_d[:].rearrange("(t p) f -> p t f", p=128),
                in_=xcur[:].rearrange("p (t f) -> p t f", f=128))

            gsb = sm.tile([128, 1], f32, tag="gsb")
            nc.vector.tensor_copy(out=gsb[:], in_=gps[:])
            nc.sync.dma_start(out=gsum_in[:], in_=gsb[:])
            nc.gpsimd.collective_compute("AllReduce", Alu.add, replica_groups=RG,
                                         ins=[gsum_in[:]], outs=[gsum_out[:]])
            gsT = sm.tile([128, 1], f32, tag="gsT")
            nc.sync.dma_start(out=gsT[:], in_=gsum_out[:])
            g1 = sm.tile([128, 1], f32, tag="g1")
            nc.vector.tensor_scalar_mul(g1[:], gsT[:], 1.0 / cfg.NREAL)
            vps = psd.tile([128, 1], f32, tag="mm")
            nc.tensor.matmul(out=vps[:], lhsT=vhw1[:], rhs=g1[:], start=True,
                             stop=True)
            g2 = sm.tile([128, 1], f32, tag="g2")
            nc.scalar.activation(out=g2[:], in_=vps[:], func=Act.Relu,
                                 bias=vhb1[:, 0:1])
            vps2 = psd.tile([1, 1], f32, tag="mm")
            nc.tensor.matmul(out=vps2[:], lhsT=vhw2[:], rhs=g2[:], start=True,
                             stop=True)
            val = sm.tile([1, 1], f32, tag="val")
            nc.vector.tensor_tensor(out=val[:], in0=vps2[:], in1=vhb2[:],
                                    op=Alu.add)
            nc.sync.dma_start(out=value_d[:], in_=val[:])

    nc.finalize()
    return nc


# ---------------------------------------------------------------- in_maps
def make_in_maps(cfg, inputs, cores):
    nf = np.asarray(inputs["node_features"], np.float32)
    nf_pad = np.zeros((cfg.NV, 3), np.float32)
    nf_pad[:cfg.NREAL] = nf
    nmask_full = np.zeros(cfg.NV, np.float32)
    nmask_full[:cfg.NREAL] = 1.0

    def f32c(x, shape=None):
        a = np.ascontiguousarray(np.asarray(x, np.float32))
        if shape is not None:
            a = a.reshape(shape)
        return a

    common = dict(
        lw3=f32c(inputs["gat_lin_w"], (3 * HID, HID)),
        lew3=f32c(inputs["gat_lin_edge_w"], (3 * HID, HID)),
        atts=f32c(inputs["gat_att_src"], (3 * H, C)),
        attd=f32c(inputs["gat_att_dst"], (3 * H, C)),
        atte=f32c(inputs["gat_att_edge"], (3 * H, C)),
        gbias=f32c(inputs["gat_bias"], (3, HID)),
        npw=f32c(inputs["np_w"]),
        npb=f32c(inputs["np_b"], (1, HID)),
        epw=f32c(inputs["ep_w"]),
        epb=f32c(inputs["ep_b"], (1, HID)),
        ahw1=f32c(inputs["ah_w1"]),
        ahb1=f32c(inputs["ah_b1"], (HID, 1)),
        ahw2=f32c(inputs["ah_w2"]),
        ahb2=f32c(inputs["ah_b2"], (1, 8)),
        vhw1=f32c(inputs["vh_w1"]),
        vhb1=f32c(inputs["vh_b1"], (HID, 1)),
        vhw2=f32c(inputs["vh_w2"]),
        vhb2=f32c(inputs["vh_b2"], (1, 1)),
    )
    in_maps = []
    for k in range(NCORES):
        cd = cores[k]
        nfk = nf_pad[k * cfg.NPC:(k + 1) * cfg.NPC]
        nmk = nmask_full[k * cfg.NPC:(k + 1) * cfg.NPC]
        m = dict(common)
        m["nf"] = np.ascontiguousarray(nfk)
        m["nmask"] = np.ascontiguousarray(
            nmk.reshape(cfg.TPC, 128).T.astype(np.float32))
        m["idxmain"] = cd["idxmain"]
        m["dloc"] = cd["dloc"]
        m["eav"] = cd["eav"]
        in_maps.append(m)
    return in_maps


def assemble_outputs(cfg, results):
    embs = np.concatenate([r["out_embs"] for r in results], 0)[:cfg.NREAL]
    logits = np.concatenate([r["out_logits"] for r in results], 0)[:cfg.NREAL]
    value = results[0]["out_value"]
    return logits.reshape(-1).astype(np.float32), value.astype(np.float32), \
        embs.astype(np.float32)


# ---------------------------------------------------------------- entry
def kernel(**inputs):
    cfg = CFG_FULL
    cores, slo, shi, B2 = host_prep(cfg, inputs["edge_index"],
                                    inputs["edge_attr"])
    nc = build_nc(cfg, slo, shi)
    in_maps = make_in_maps(cfg, inputs, cores)
    from concourse.bass_utils import run_bass_kernel_spmd
    res = run_bass_kernel_spmd(nc, in_maps, core_ids=list(range(NCORES)),
                               trace=bool(int(os.environ.get("KBENCH_TRACE", "0"))))
    out = assemble_outputs(cfg, res.results)
    if res.exec_time_ns is not None:
        print(f"HW exec time: {res.exec_time_ns} ns "
              f"(mean {res.mean_exec_time_ns} ns)")
    return out


# revision 16
# speedup vs baseline: 1.1803x; 1.1803x over previous
"""Trainium2 Bass kernel for nn_DiscreteGNNPolicy (3-layer edge-featured GAT).

Strategy (graph/data parallel over 8 NeuronCores):
  - Nodes are sharded by contiguous range: 6272 virtual nodes per core
    (50176 total, >= 50000 real).  Edges are routed on the host to the core
    owning their dst node, sorted by dst, and packed into 128-edge blocks
    grouped by 32-node windows (host does routing/permutation only).
  - Per layer, each core densely computes a per-node "T row"
    [xs interleaved with ones (132) | a_src (4) | a_dst (4) | pad to 256]
    (fp16) for its node slice; slices are AllGathered into a full T table
    in DRAM.
  - Edge phase: per group (128 nodes = 4 windows), dma_gather fetches
    T[src] rows (512B each; the int16-index limit is handled by routing
    each 128-edge block to be homogeneous in src < 32768 vs >= 32768 and
    gathering the "hi" blocks from an offset view of the table), and a
    second dma_gather fetches the dst rows' a_dst columns from the LOCAL
    slice (dst is always local).  exp(leaky_relu(logit)) is folded into a
    sparse selection matrix A[e,(h,j)] = p[e,h] * [dst_local[e]==j], and
    the segment-softmax numerator+denominator are ONE PE matmul per
    128-edge block accumulating into PSUM:
        out[(h,j), c] += sum_e A[e,(h,j)] * G[e,c]
    (the interleaved ones-columns of G yield the softmax denominators).
  - Self loops (PyG fill_value='mean') are handled densely per node from
    segment statistics (cnt, sum of edge_attr) computed with the same
    indicator matmuls in layer 0.
  - Softmax never subtracts the segment max: logits are O(1) here, and
    exp(a)/sum(exp(a)) is mathematically identical.

kernel(**inputs) takes the FULL inputs and returns
(action_logits [N*8], value [1,1], node_embs [N,128]) like the reference.
"""

import math
import os

import numpy as np

# ---------------------------------------------------------------- config
N_REAL = 50000
E_REAL = 1600000
H = 4
C = 32
HID = 128
L = 3
NEG = 0.2
NCORES = 8
TROW = 256       # fp16 elems per T row (512B); 0:132 xs+ones, 132:136 asrc,
                 # 136:140 adst, rest pad
TUSE = 140
LOSPLIT = 32768  # dma_gather int16 index limit


class Cfg:
    def __init__(self, n_real, npc, nw=32):
        self.NREAL = n_real
        self.NPC = npc                  # nodes per core (multiple of 128)
        self.NV = NCORES * npc          # virtual (padded) node count
        self.NW = nw                    # window = nodes per PSUM row set
        self.WPC = npc // nw            # windows per core
        self.TPC = npc // 128           # 128-node tiles (= groups) per core
        self.GWIN = 128 // nw           # windows per group (4)
        assert npc % 128 == 0 and nw * self.GWIN == 128


CFG_FULL = Cfg(N_REAL, 6272)


def _wrap16(slot_arr):
    """[128, B] per-slot values -> dma_gather wrapped idx layout [128, B*8]
    (data in first 16 partitions; idx i of a call at (i%16, bstart*8+i//16),
    which for 128-aligned blocks reduces to a per-block transform)."""
    p128, B = slot_arr.shape
    assert p128 == 128
    w = slot_arr.reshape(8, 16, B).transpose(1, 2, 0).reshape(16, B * 8)
    return np.ascontiguousarray(np.tile(w, (8, 1)))


# ---------------------------------------------------------------- host prep
def host_prep(cfg, edge_index, edge_attr):
    """Pure routing/permutation/padding. Returns per-core slot arrays and the
    shared (slo, shi) block schedules."""
    src = np.asarray(edge_index[0]).astype(np.int64)
    dst = np.asarray(edge_index[1]).astype(np.int64)
    ea = np.asarray(edge_attr, np.float32)

    order = np.argsort(dst, kind="stable")
    src_s, dst_s, ea_s = src[order], dst[order], ea[order]
    lo_s = src_s < LOSPLIT
    gwin = dst_s // cfg.NW
    nwin_g = cfg.NV // cfg.NW
    # counts per (window, lo/hi)
    cnt_lo = np.bincount(gwin[lo_s], minlength=nwin_g).reshape(NCORES, cfg.WPC)
    cnt_hi = np.bincount(gwin[~lo_s], minlength=nwin_g).reshape(NCORES, cfg.WPC)
    slo = np.maximum(1, (cnt_lo + 127) // 128).max(axis=0).astype(np.int64)
    shi = ((cnt_hi + 127) // 128).max(axis=0).astype(np.int64)
    if cfg.NV <= LOSPLIT:
        assert shi.sum() == 0

    GW = cfg.GWIN
    nbLo_g = slo.reshape(-1, GW).sum(1)
    nbHi_g = shi.reshape(-1, GW).sum(1)
    gbase = np.concatenate([[0], np.cumsum(nbLo_g + nbHi_g)])
    # per-window block bases in the global slot order
    lo_base = np.zeros(cfg.WPC, np.int64)
    hi_base = np.zeros(cfg.WPC, np.int64)
    for g in range(cfg.TPC):
        accl = gbase[g]
        for wi in range(GW):
            lo_base[g * GW + wi] = accl
            accl += slo[g * GW + wi]
        acch = gbase[g] + nbLo_g[g]
        for wi in range(GW):
            hi_base[g * GW + wi] = acch
            acch += shi[g * GW + wi]
    B2 = int(gbase[-1])

    # sort window-local edges: within each window put lo edges first? No —
    # separate placement per lo/hi below.
    starts_all = np.concatenate([[0], np.cumsum(np.bincount(gwin, minlength=nwin_g))])

    cores = []
    for k in range(NCORES):
        srcv = np.zeros((128, B2), np.int16)
        dstv = np.zeros((128, B2), np.int16)
        dloc = np.full((128, B2), cfg.NW, np.float16)
        eav = np.zeros((128, B2, 3), np.float16)
        for w in range(cfg.WPC):
            g = k * cfg.WPC + w
            s, e = starts_all[g], starts_all[g + 1]
            if e == s:
                continue
            wsrc, wdst, wea = src_s[s:e], dst_s[s:e], ea_s[s:e]
            wlo = wsrc < LOSPLIT
            for sel, base, sub in ((wlo, lo_base[w], 0),
                                   (~wlo, hi_base[w], LOSPLIT)):
                c = int(sel.sum())
                if c == 0:
                    continue
                sl = np.arange(c)
                p = sl % 128
                b = base + sl // 128
                srcv[p, b] = (wsrc[sel] - sub).astype(np.int16)
                dstv[p, b] = (wdst[sel] - k * cfg.NPC).astype(np.int16)
                dloc[p, b] = (wdst[sel] - g * cfg.NW).astype(np.float16)
                eav[p, b, 0] = wea[sel, 0]
                eav[p, b, 1] = wea[sel, 1]
                eav[p, b, 2] = 1.0
        cores.append(dict(idxmain=_wrap16(srcv),
                          dloc=dloc, eav=eav.reshape(128, B2 * 3)))
    return cores, slo, shi, B2


# ---------------------------------------------------------------- builder
def build_nc(cfg, slo, shi):
    import concourse.bacc as bacc
    import concourse.tile as tile
    from concourse import bass, mybir
    from concourse.masks import make_identity

    f32 = mybir.dt.float32
    f16 = mybir.dt.float16
    i16 = mybir.dt.int16
    i32 = mybir.dt.int32
    Alu = mybir.AluOpType
    Act = mybir.ActivationFunctionType
    Ax = mybir.AxisListType

    NPC, NV, NW, WPC, TPC, GWIN = (cfg.NPC, cfg.NV, cfg.NW, cfg.WPC,
                                   cfg.TPC, cfg.GWIN)
    GW = GWIN
    nbLo_g = slo.reshape(-1, GW).sum(1)
    nbHi_g = shi.reshape(-1, GW).sum(1)
    gbase = np.concatenate([[0], np.cumsum(nbLo_g + nbHi_g)]).astype(int)
    B2 = int(gbase[-1])

    nc = bacc.Bacc("TRN2", target_bir_lowering=False, debug=False,
                   num_devices=NCORES, num_swdge_queues=4)
    RG = [list(range(NCORES))]

    # ---------- I/O -------------------------------------------------------
    nf_d = nc.declare_dram_parameter("nf", [NPC, 3], f32, isOutput=False)
    nmask_d = nc.declare_dram_parameter("nmask", [128, TPC], f32, isOutput=False)
    idxm_d = nc.declare_dram_parameter("idxmain", [128, B2 * 8], i16, isOutput=False)
    dloc_d = nc.declare_dram_parameter("dloc", [128, B2], f16, isOutput=False)
    eav_d = nc.declare_dram_parameter("eav", [128, B2 * 3], f16, isOutput=False)
    lw3_d = nc.declare_dram_parameter("lw3", [3 * HID, HID], f32, isOutput=False)
    lew3_d = nc.declare_dram_parameter("lew3", [3 * HID, HID], f32, isOutput=False)
    atts_d = nc.declare_dram_parameter("atts", [3 * H, C], f32, isOutput=False)
    attd_d = nc.declare_dram_parameter("attd", [3 * H, C], f32, isOutput=False)
    atte_d = nc.declare_dram_parameter("atte", [3 * H, C], f32, isOutput=False)
    gbias_d = nc.declare_dram_parameter("gbias", [3, HID], f32, isOutput=False)
    npw_d = nc.declare_dram_parameter("npw", [3, HID], f32, isOutput=False)
    npb_d = nc.declare_dram_parameter("npb", [1, HID], f32, isOutput=False)
    epw_d = nc.declare_dram_parameter("epw", [2, HID], f32, isOutput=False)
    epb_d = nc.declare_dram_parameter("epb", [1, HID], f32, isOutput=False)
    ahw1_d = nc.declare_dram_parameter("ahw1", [HID, HID], f32, isOutput=False)
    ahb1_d = nc.declare_dram_parameter("ahb1", [HID, 1], f32, isOutput=False)
    ahw2_d = nc.declare_dram_parameter("ahw2", [HID, 8], f32, isOutput=False)
    ahb2_d = nc.declare_dram_parameter("ahb2", [1, 8], f32, isOutput=False)
    vhw1_d = nc.declare_dram_parameter("vhw1", [HID, HID], f32, isOutput=False)
    vhb1_d = nc.declare_dram_parameter("vhb1", [HID, 1], f32, isOutput=False)
    vhw2_d = nc.declare_dram_parameter("vhw2", [HID, 1], f32, isOutput=False)
    vhb2_d = nc.declare_dram_parameter("vhb2", [1, 1], f32, isOutput=False)

    embs_d = nc.declare_dram_parameter("out_embs", [NPC, HID], f32, isOutput=True)
    logit_d = nc.declare_dram_parameter("out_logits", [NPC, 8], f32, isOutput=True)
    value_d = nc.declare_dram_parameter("out_value", [1, 1], f32, isOutput=True)

    # ---------- internal DRAM --------------------------------------------
    tslice = [nc.dram_tensor(f"tslice{l}", [NPC, TROW], f16) for l in range(L)]
    tfull = [nc.dram_tensor(f"tfull{l}", [NV, TROW], f16, addr_space="Shared")
             for l in range(L)]
    aetab = [nc.dram_tensor(f"aetab{l}", [128, B2 * 4], f16) for l in range(L)]
    bscr = nc.dram_tensor("bscr", [4, TROW], f32)
    adscr = nc.dram_tensor("adscr", [TPC, 512], f16)
    gsum_in = nc.dram_tensor("gsum_in", [HID, 1], f32)
    gsum_out = nc.dram_tensor("gsum_out", [HID, 1], f32, addr_space="Shared")

    with tile.TileContext(nc) as tc:
        with (tc.tile_pool(name="per", bufs=1) as per,
              tc.tile_pool(name="grp", bufs=3) as grp,
              tc.tile_pool(name="small", bufs=2) as sm,
              tc.tile_pool(name="psw", bufs=2, space="PSUM") as psw,
              tc.tile_pool(name="psd", bufs=2, space="PSUM") as psd,
              tc.tile_pool(name="psg", bufs=1, space="PSUM") as psg):

            # ---------------- persistent inputs ------------------------
            dloc = per.tile([128, B2], f16)
            nmask = per.tile([128, TPC], f32)
            nc.sync.dma_start(out=dloc[:], in_=dloc_d[:])
            nc.sync.dma_start(out=nmask[:], in_=nmask_d[:])

            ident = per.tile([128, 128], f32)
            make_identity(nc, ident[:])
            iot32 = per.tile([128, NW], i32)
            nc.gpsimd.iota(iot32[:], pattern=[[1, NW]], base=0,
                           channel_multiplier=0)
            iota16 = per.tile([128, NW], f16)
            nc.vector.tensor_copy(out=iota16[:], in_=iot32[:])

            lw_sb = [per.tile([128, 128], f32, tag=f"lw{l}", name=f"lw{l}")
                     for l in range(L)]
            lew_sb = [per.tile([128, 128], f32, tag=f"lew{l}", name=f"lew{l}")
                      for l in range(L)]
            for l in range(L):
                nc.sync.dma_start(out=lw_sb[l][:], in_=lw3_d[l * 128:(l + 1) * 128, :])
                nc.sync.dma_start(out=lew_sb[l][:], in_=lew3_d[l * 128:(l + 1) * 128, :])
            attsb = per.tile([128, 3 * H * C], f32, tag="attsb")
            attdb = per.tile([128, 3 * H * C], f32, tag="attdb")
            atteb = per.tile([128, 3 * H * C], f32, tag="atteb")
            for r in range(3 * H):
                for dsttile, srcd in ((attsb, atts_d), (attdb, attd_d), (atteb, atte_d)):
                    nc.sync.dma_start(
                        out=dsttile[:, r * C:(r + 1) * C],
                        in_=srcd[r:r + 1, :].partition_broadcast(128))
            gbb = [per.tile([128, HID], f32, tag=f"gbb{l}", name=f"gbb{l}")
                   for l in range(L)]
            for l in range(L):
                nc.sync.dma_start(out=gbb[l][:],
                                  in_=gbias_d[l:l + 1, :].partition_broadcast(128))
            npw_sb = per.tile([3, HID], f32)
            npb_sb = per.tile([1, HID], f32)
            epw_sb = per.tile([2, HID], f32)
            epb_sb = per.tile([1, HID], f32)
            nc.sync.dma_start(out=npw_sb[:], in_=npw_d[:])
            nc.sync.dma_start(out=npb_sb[:], in_=npb_d[:])
            nc.sync.dma_start(out=epw_sb[:], in_=epw_d[:])
            nc.sync.dma_start(out=epb_sb[:], in_=epb_d[:])
            ahw1 = per.tile([HID, HID], f32)
            ahb1 = per.tile([HID, 1], f32)
            ahw2 = per.tile([HID, 8], f32)
            ahb2b = per.tile([128, 8], f32)
            vhw1 = per.tile([HID, HID], f32)
            vhb1 = per.tile([HID, 1], f32)
            vhw2 = per.tile([HID, 1], f32)
            vhb2 = per.tile([1, 1], f32)
            nc.sync.dma_start(out=ahw1[:], in_=ahw1_d[:])
            nc.sync.dma_start(out=ahb1[:], in_=ahb1_d[:])
            nc.sync.dma_start(out=ahw2[:], in_=ahw2_d[:])
            nc.sync.dma_start(out=ahb2b[:], in_=ahb2_d[:].partition_broadcast(128))
            nc.sync.dma_start(out=vhw1[:], in_=vhw1_d[:])
            nc.sync.dma_start(out=vhb1[:], in_=vhb1_d[:])
            nc.sync.dma_start(out=vhw2[:], in_=vhw2_d[:])
            nc.sync.dma_start(out=vhb2[:], in_=vhb2_d[:])

            # ---------------- fold weights -----------------------------
            we_sb = per.tile([128, L * H], f32)
            wcat = [per.tile([128, TUSE], f32, tag=f"wcat{l}", name=f"wcat{l}")
                    for l in range(L)]
            for l in range(L):
                nc.vector.memset(wcat[l][:], 0.0)
                nc.vector.tensor_copy(
                    out=wcat[l][:, 0:132].rearrange("p (h c) -> p h c", c=33)[:, :, 0:32],
                    in_=lw_sb[l][:].rearrange("p (h c) -> p h c", c=32))
                for h in range(H):
                    r = l * H + h
                    tmphc = sm.tile([128, C], f32, tag="tmphc")
                    nc.vector.tensor_tensor(
                        out=tmphc[:], in0=lw_sb[l][:, h * C:(h + 1) * C],
                        in1=attsb[:, r * C:(r + 1) * C], op=Alu.mult)
                    nc.vector.tensor_reduce(out=wcat[l][:, 132 + h:133 + h],
                                            in_=tmphc[:], axis=Ax.X, op=Alu.add)
                    tmphc = sm.tile([128, C], f32, tag="tmphc")
                    nc.vector.tensor_tensor(
                        out=tmphc[:], in0=lw_sb[l][:, h * C:(h + 1) * C],
                        in1=attdb[:, r * C:(r + 1) * C], op=Alu.mult)
                    nc.vector.tensor_reduce(out=wcat[l][:, 136 + h:137 + h],
                                            in_=tmphc[:], axis=Ax.X, op=Alu.add)
                    tmphc = sm.tile([128, C], f32, tag="tmphc")
                    nc.vector.tensor_tensor(
                        out=tmphc[:], in0=lew_sb[l][:, h * C:(h + 1) * C],
                        in1=atteb[:, r * C:(r + 1) * C], op=Alu.mult)
                    nc.vector.tensor_reduce(out=we_sb[:, r:r + 1],
                                            in_=tmphc[:], axis=Ax.X, op=Alu.add)

            tp_ps = psd.tile([128, 2], f32, tag="mm")
            nc.tensor.transpose(out=tp_ps[:], in_=epw_sb[:], identity=ident[0:2, 0:2])
            epwT = per.tile([128, 2], f32)
            nc.vector.tensor_copy(out=epwT[:], in_=tp_ps[:])
            tp_ps1 = psd.tile([128, 1], f32, tag="mm")
            nc.tensor.transpose(out=tp_ps1[:], in_=epb_sb[:], identity=ident[0:1, 0:1])
            epbT = per.tile([128, 1], f32)
            nc.vector.tensor_copy(out=epbT[:], in_=tp_ps1[:])
            m2sb = sm.tile([2, L * H], f32, tag="m2sb")
            clsb = sm.tile([1, L * H], f32, tag="clsb")
            for l in range(L):
                mps = psd.tile([2, H], f32, tag="mm")
                nc.tensor.matmul(out=mps[:], lhsT=epwT[:],
                                 rhs=we_sb[:, l * H:(l + 1) * H], start=True, stop=True)
                nc.vector.tensor_copy(out=m2sb[:, l * H:(l + 1) * H], in_=mps[:])
                cps = psd.tile([1, H], f32, tag="mm")
                nc.tensor.matmul(out=cps[:], lhsT=epbT[:],
                                 rhs=we_sb[:, l * H:(l + 1) * H], start=True, stop=True)
                nc.vector.tensor_copy(out=clsb[:, l * H:(l + 1) * H], in_=cps[:])
            nc.sync.dma_start(out=bscr[0:1, 0:L * H], in_=m2sb[0:1, :])
            nc.sync.dma_start(out=bscr[1:2, 0:L * H], in_=m2sb[1:2, :])
            nc.sync.dma_start(out=bscr[2:3, 0:L * H], in_=clsb[:])

            tp_ps3 = psd.tile([128, 3], f32, tag="mm")
            nc.tensor.transpose(out=tp_ps3[:], in_=npw_sb[:],
                                identity=ident[0:3, 0:3])
            npwT = per.tile([128, 3], f32)
            nc.vector.tensor_copy(out=npwT[:], in_=tp_ps3[:])
            m0ps = psd.tile([3, TUSE], f32, tag="mm")
            nc.tensor.matmul(out=m0ps[:], lhsT=npwT[:], rhs=wcat[0][:],
                             start=True, stop=True)
            m0sb = per.tile([3, TUSE], f32)
            nc.vector.tensor_copy(out=m0sb[:], in_=m0ps[:])
            tp_ps1b = psd.tile([128, 1], f32, tag="mm")
            nc.tensor.transpose(out=tp_ps1b[:], in_=npb_sb[:],
                                identity=ident[0:1, 0:1])
            npbT = per.tile([128, 1], f32)
            nc.vector.tensor_copy(out=npbT[:], in_=tp_ps1b[:])
            b0ps = psd.tile([1, TUSE], f32, tag="mm")
            nc.tensor.matmul(out=b0ps[:], lhsT=npbT[:], rhs=wcat[0][:],
                             start=True, stop=True)
            b0sb = sm.tile([1, TUSE], f32, tag="b0sb")
            nc.vector.tensor_copy(out=b0sb[:], in_=b0ps[:])
            nc.sync.dma_start(out=bscr[3:4, 0:TUSE], in_=b0sb[:])

            m2b0 = per.tile([128, L * H], f32)
            m2b1 = per.tile([128, L * H], f32)
            cb12 = per.tile([128, L * H], f32)
            bias0b = per.tile([128, TUSE], f32)
            nc.sync.dma_start(out=m2b0[:], in_=bscr[0:1, 0:L * H].partition_broadcast(128))
            nc.sync.dma_start(out=m2b1[:], in_=bscr[1:2, 0:L * H].partition_broadcast(128))
            nc.sync.dma_start(out=cb12[:], in_=bscr[2:3, 0:L * H].partition_broadcast(128))
            nc.sync.dma_start(out=bias0b[:], in_=bscr[3:4, 0:TUSE].partition_broadcast(128))

            # ---------------- aetab ------------------------------------
            CHK = 256
            NCHUNK = max(1, math.ceil(B2 / CHK))
            for l in range(L):
                for ci in range(NCHUNK):
                    c0 = ci * CHK
                    c1 = min(B2, c0 + CHK)
                    cB = c1 - c0
                    if cB <= 0:
                        continue
                    eavc = sm.tile([128, CHK * 3], f16, tag="eavc", bufs=1)
                    nc.sync.dma_start(out=eavc[:, :cB * 3],
                                      in_=eav_d[:, c0 * 3:c1 * 3])
                    ea3 = eavc[:, :cB * 3].rearrange("p (b e) -> p b e", e=3)
                    t1 = sm.tile([128, CHK * 4], f32, tag="aet1", bufs=1)
                    t2 = sm.tile([128, CHK * 4], f32, tag="aet2", bufs=1)
                    nc.vector.tensor_tensor(
                        out=t1[:, :cB * 4].rearrange("p (b h) -> p b h", h=4),
                        in0=ea3[:, :, 0:1].to_broadcast([128, cB, 4]),
                        in1=m2b0[:, l * 4:(l + 1) * 4].unsqueeze(1).to_broadcast([128, cB, 4]),
                        op=Alu.mult)
                    nc.vector.tensor_tensor(
                        out=t2[:, :cB * 4].rearrange("p (b h) -> p b h", h=4),
                        in0=ea3[:, :, 1:2].to_broadcast([128, cB, 4]),
                        in1=m2b1[:, l * 4:(l + 1) * 4].unsqueeze(1).to_broadcast([128, cB, 4]),
                        op=Alu.mult)
                    nc.vector.tensor_tensor(out=t1[:, :cB * 4], in0=t1[:, :cB * 4],
                                            in1=t2[:, :cB * 4], op=Alu.add)
                    ae16 = sm.tile([128, CHK * 4], f16, tag="ae16", bufs=1)
                    nc.vector.tensor_tensor(
                        out=ae16[:, :cB * 4].rearrange("p (b h) -> p b h", h=4),
                        in0=t1[:, :cB * 4].rearrange("p (b h) -> p b h", h=4),
                        in1=cb12[:, l * 4:(l + 1) * 4].unsqueeze(1).to_broadcast([128, cB, 4]),
                        op=Alu.add)
                    nc.sync.dma_start(out=aetab[l][:, c0 * 4:c1 * 4],
                                      in_=ae16[:, :cB * 4])

            # ---------------- persistent state -------------------------
            tloc = per.tile([128, TPC * TUSE], f16)
            xcur = per.tile([128, TPC * 128], f32)
            la_sb = per.tile([128, TPC * L * H], f32)

            # ---------------- dense phase ------------------------------
            def dense_phase(l):
                for t in range(TPC):
                    TL = tloc[:, t * TUSE:(t + 1) * TUSE]
                    if l == 0:
                        nfT_ps = psd.tile([3, 128], f32, tag="mm")
                        nc.tensor.transpose(out=nfT_ps[:],
                                            in_=nfsb[:, t * 3:(t + 1) * 3],
                                            identity=ident[:])
                        nfT = sm.tile([3, 128], f32, tag="nfT")
                        nc.vector.tensor_copy(out=nfT[:], in_=nfT_ps[:])
                        dps = psd.tile([128, TUSE], f32, tag="mm")
                        nc.tensor.matmul(out=dps[:], lhsT=nfT[:], rhs=m0sb[:],
                                         start=True, stop=True)
                        nc.vector.tensor_tensor(out=TL, in0=dps[:], in1=bias0b[:],
                                                op=Alu.add)
                    else:
                        xT_ps = psd.tile([128, 128], f32, tag="mm")
                        nc.tensor.transpose(out=xT_ps[:],
                                            in_=xcur[:, t * 128:(t + 1) * 128],
                                            identity=ident[:])
                        xT = sm.tile([128, 128], f32, tag="xT")
                        nc.vector.tensor_copy(out=xT[:], in_=xT_ps[:])
                        dps = psd.tile([128, TUSE], f32, tag="mm")
                        nc.tensor.matmul(out=dps[:], lhsT=xT[:], rhs=wcat[l][:],
                                         start=True, stop=True)
                        nc.vector.tensor_copy(out=TL, in_=dps[:])
                    nc.vector.memset(
                        TL[:, 0:132].rearrange("p (h c) -> p h c", c=33)[:, :, 32], 1.0)
                nc.sync.dma_start(
                    out=tslice[l][:].rearrange("(t p) f -> p t f", p=128)[:, :, 0:TUSE],
                    in_=tloc[:].rearrange("p (t f) -> p t f", f=TUSE))
                nc.gpsimd.collective_compute(
                    "AllGather", Alu.bypass, replica_groups=RG,
                    ins=[tslice[l][:]], outs=[tfull[l][:]])

            nfsb = per.tile([128, TPC * 3], f32)
            nc.sync.dma_start(out=nfsb[:],
                              in_=nf_d[:].rearrange("(t p) f -> p t f", p=128))

            # zero the pad columns of every tslice once (the gathers read
            # full 512B rows; uninitialized DRAM would be NaN in sim)
            zpad = per.tile([128, TROW - TUSE], f16)
            nc.vector.memset(zpad[:], 0.0)
            for l in range(L):
                nc.sync.dma_start(
                    out=tslice[l][:].rearrange("(t p) f -> p t f", p=128)[:, :, TUSE:TROW],
                    in_=zpad[:].unsqueeze(1).to_broadcast([128, TPC, TROW - TUSE]))

            dense_phase(0)

            # ---------------- edge phase -------------------------------
            boff_lo = []
            boff_hi = []
            for g in range(TPC):
                accl = 0
                acch = int(nbLo_g[g])
                ls, hs = [], []
                for wi in range(GW):
                    ls.append(accl)
                    hs.append(acch)
                    accl += int(slo[g * GW + wi])
                    acch += int(shi[g * GW + wi])
                boff_lo.append(ls)
                boff_hi.append(hs)
            qrr = [0]
            for l in range(L):
                for g in range(TPC):
                    w0 = g * GW
                    g0 = int(gbase[g])
                    nbLo = int(nbLo_g[g])
                    nbHi = int(nbHi_g[g])
                    nbG = nbLo + nbHi
                    # ---- gathers (<=1024 idx per call, 4 SWDGE queues) ----
                    Gt = grp.tile([128, nbG * TROW], f16, tag="Gt")
                    idxg = grp.tile([128, nbG * 8], i16, tag="idxg")
                    nc.sync.dma_start(out=idxg[:],
                                      in_=idxm_d[:, g0 * 8:(g0 + nbG) * 8])
                    for c0 in range(0, nbLo, 8):
                        nb = min(8, nbLo - c0)
                        nc.gpsimd.dma_gather(
                            out_ap=Gt[:, (c0) * TROW:(c0 + nb) * TROW]
                                .rearrange("p (b f) -> p b f", f=TROW),
                            in_ap=tfull[l][:, :],
                            idxs_ap=idxg[:, c0 * 8:(c0 + nb) * 8],
                            num_idxs=nb * 128, num_idxs_reg=nb * 128,
                            elem_size=TROW, queue_num=qrr[0] % 4)
                        qrr[0] += 1
                    for c0 in range(0, nbHi, 8):
                        nb = min(8, nbHi - c0)
                        nc.gpsimd.dma_gather(
                            out_ap=Gt[:, (nbLo + c0) * TROW:(nbLo + c0 + nb) * TROW]
                                .rearrange("p (b f) -> p b f", f=TROW),
                            in_ap=tfull[l][LOSPLIT:NV, :],
                            idxs_ap=idxg[:, (nbLo + c0) * 8:(nbLo + c0 + nb) * 8],
                            num_idxs=nb * 128, num_idxs_reg=nb * 128,
                            elem_size=TROW, queue_num=qrr[0] % 4)
                        qrr[0] += 1
                    aet = grp.tile([128, nbG * 4], f16, tag="aet")
                    nc.sync.dma_start(out=aet[:],
                                      in_=aetab[l][:, g0 * 4:(g0 + nbG) * 4])
                    # ---- a_dst row broadcast (local tile -> DRAM -> bcast) ----
                    TLg = tloc[:, g * TUSE:(g + 1) * TUSE]
                    nc.sync.dma_start(
                        out=adscr[g, :].rearrange("(p h) -> p h", h=4),
                        in_=TLg[:, 136:140])
                    adb = grp.tile([128, 512], f16, tag="adb")
                    nc.sync.dma_start(out=adb[:],
                                      in_=adscr[g:g + 1, :].partition_broadcast(128))
                    # ---- indicator (needed for a_dst select and A) ----
                    Ig = grp.tile([128, nbG * NW], f16, tag="Ig")
                    nc.vector.tensor_tensor(
                        out=Ig[:].rearrange("p (b j) -> p b j", j=NW),
                        in0=iota16[:].unsqueeze(1).to_broadcast([128, nbG, NW]),
                        in1=dloc[:, g0:g0 + nbG].unsqueeze(2).to_broadcast([128, nbG, NW]),
                        op=Alu.is_equal)
                    # ---- per-edge logits ----
                    sg = grp.tile([128, nbG * 4], f32, tag="sg")
                    G3 = Gt[:].rearrange("p (b f) -> p b f", f=TROW)
                    nc.vector.tensor_tensor(
                        out=sg[:].rearrange("p (b h) -> p b h", h=4),
                        in0=G3[:, :, 132:136],
                        in1=aet[:].rearrange("p (b h) -> p b h", h=4), op=Alu.add)
                    # a_dst select: t = I (*) adst_row ; reduce over j
                    boff = 0
                    for wi in range(GW):
                        nbW = int(slo[w0 + wi]) + int(shi[w0 + wi])
                        blks = ([boff_lo[g][wi] + j for j in range(int(slo[w0 + wi]))] +
                                [boff_hi[g][wi] + j for j in range(int(shi[w0 + wi]))])
                        tsel = grp.tile([128, 8 * 4 * NW], f16, tag="tsel", bufs=3)
                        adp = grp.tile([128, 8 * 4], f32, tag="adp", bufs=3)
                        for ci, gb in enumerate(blks):
                            pass
                        # build t for this window's blocks (they are two
                        # contiguous runs; do each run in one op)
                        runs = [(boff_lo[g][wi], int(slo[w0 + wi])),
                                (boff_hi[g][wi], int(shi[w0 + wi]))]
                        for rb, rn in runs:
                            if rn == 0:
                                continue
                            nc.vector.tensor_tensor(
                                out=tsel[:, :rn * 4 * NW].rearrange(
                                    "p (b h j) -> p b h j", h=4, j=NW),
                                in0=Ig[:, rb * NW:(rb + rn) * NW]
                                    .rearrange("p (b j) -> p b j", j=NW)
                                    .unsqueeze(2).to_broadcast([128, rn, 4, NW]),
                                in1=adb[:, wi * 128:(wi + 1) * 128]
                                    .rearrange("p (j h) -> p j h", h=4)
                                    .transpose([0, 2, 1]).unsqueeze(1)
                                    .to_broadcast([128, rn, 4, NW]),
                                op=Alu.mult)
                            nc.vector.tensor_reduce(
                                out=adp[:, :rn * 4].rearrange("p (b h) -> p b h", h=4),
                                in_=tsel[:, :rn * 4 * NW].rearrange(
                                    "p (b h j) -> p b h j", h=4, j=NW),
                                axis=Ax.X, op=Alu.add)
                            nc.vector.tensor_tensor(
                                out=sg[:, rb * 4:(rb + rn) * 4],
                                in0=sg[:, rb * 4:(rb + rn) * 4],
                                in1=adp[:, :rn * 4], op=Alu.add)
                    lk = grp.tile([128, nbG * 4], f32, tag="lk")
                    lk = grp.tile([128, nbG * 4], f32, tag="lk")
                    nc.vector.tensor_scalar_mul(lk[:], sg[:], NEG)
                    nc.vector.tensor_tensor(out=sg[:], in0=sg[:], in1=lk[:],
                                            op=Alu.max)
                    pg = grp.tile([128, nbG * 4], f16, tag="pg")
                    nc.scalar.activation(out=pg[:], in_=sg[:], func=Act.Exp)
                    # ---- indicator + A ----
                    Ig = grp.tile([128, nbG * NW], f16, tag="Ig")
                    nc.vector.tensor_tensor(
                        out=Ig[:].rearrange("p (b j) -> p b j", j=NW),
                        in0=iota16[:].unsqueeze(1).to_broadcast([128, nbG, NW]),
                        in1=dloc[:, g0:g0 + nbG].unsqueeze(2).to_broadcast([128, nbG, NW]),
                        op=Alu.is_equal)
                    Ag = grp.tile([128, nbG * 4 * NW], f16, tag="Ag")
                    nc.vector.tensor_tensor(
                        out=Ag[:].rearrange("p (b h j) -> p b h j", h=4, j=NW),
                        in0=Ig[:].rearrange("p (b j) -> p b j", j=NW)
                            .unsqueeze(2).to_broadcast([128, nbG, 4, NW]),
                        in1=pg[:].rearrange("p (b h) -> p b h", h=4)
                            .unsqueeze(3).to_broadcast([128, nbG, 4, NW]),
                        op=Alu.mult)

                    if l == 0:
                        eavg = grp.tile([128, nbG * 3], f16, tag="eavg")
                        nc.sync.dma_start(out=eavg[:],
                                          in_=eav_d[:, g0 * 3:(g0 + nbG) * 3])
                        stt = grp.tile([128, 3], f32, tag="stt")
                    agg = grp.tile([128, 132], f32, tag="agg")
                    for wi in range(GW):
                        w = w0 + wi
                        blocks = ([boff_lo[g][wi] + j for j in range(int(slo[w]))] +
                                  [boff_hi[g][wi] + j for j in range(int(shi[w]))])
                        pw = psw.tile([4 * NW, 136], f32, tag="pw")
                        for bi, gb in enumerate(blocks):
                            nc.tensor.matmul(
                                out=pw[:], lhsT=Ag[:, gb * 128:(gb + 1) * 128],
                                rhs=Gt[:, gb * TROW:gb * TROW + 136],
                                start=(bi == 0), stop=(bi == len(blocks) - 1))
                        for h in range(H):
                            dst_ap = agg[wi * NW:(wi + 1) * NW, h * 33:h * 33 + 33]
                            src_ap = pw[h * NW:(h + 1) * NW, h * 33:h * 33 + 33]
                            if h % 2 == 0:
                                nc.scalar.copy(out=dst_ap, in_=src_ap)
                            else:
                                nc.vector.tensor_copy(out=dst_ap, in_=src_ap)
                        if l == 0:
                            pst = psw.tile([NW, 3], f32, tag="pst", bufs=1)
                            for bi, gb in enumerate(blocks):
                                nc.tensor.matmul(
                                    out=pst[:], lhsT=Ig[:, gb * NW:(gb + 1) * NW],
                                    rhs=eavg[:, gb * 3:gb * 3 + 3],
                                    start=(bi == 0), stop=(bi == len(blocks) - 1))
                            nc.vector.tensor_copy(
                                out=stt[wi * NW:(wi + 1) * NW, :], in_=pst[:])

                    # ---- group epilogue ----
                    TL = tloc[:, g * TUSE:(g + 1) * TUSE]
                    if l == 0:
                        cmax = sm.tile([128, 1], f32, tag="cmax")
                        nc.vector.tensor_scalar_max(cmax[:], stt[:, 2:3], 1.0)
                        rec = sm.tile([128, 1], f32, tag="rec")
                        nc.vector.reciprocal(out=rec[:], in_=cmax[:])
                        mea = sm.tile([128, 2], f32, tag="mea")
                        nc.vector.tensor_tensor(out=mea[:], in0=stt[:, 0:2],
                                                in1=rec[:].to_broadcast([128, 2]),
                                                op=Alu.mult)
                        lt1 = sm.tile([128, L * H], f32, tag="lt1")
                        lt2 = sm.tile([128, L * H], f32, tag="lt2")
                        nc.vector.tensor_tensor(
                            out=lt1[:], in0=mea[:, 0:1].to_broadcast([128, L * H]),
                            in1=m2b0[:], op=Alu.mult)
                        nc.vector.tensor_tensor(
                            out=lt2[:], in0=mea[:, 1:2].to_broadcast([128, L * H]),
                            in1=m2b1[:], op=Alu.mult)
                        nc.vector.tensor_tensor(out=lt1[:], in0=lt1[:], in1=lt2[:],
                                                op=Alu.add)
                        nc.vector.tensor_tensor(
                            out=la_sb[:, g * L * H:(g + 1) * L * H],
                            in0=lt1[:], in1=cb12[:], op=Alu.add)
                    slp = sm.tile([128, 4], f32, tag="slp")
                    nc.vector.tensor_tensor(out=slp[:], in0=TL[:, 132:136],
                                            in1=TL[:, 136:140], op=Alu.add)
                    nc.vector.tensor_tensor(
                        out=slp[:], in0=slp[:],
                        in1=la_sb[:, g * L * H + l * 4:g * L * H + l * 4 + 4],
                        op=Alu.add)
                    slk = sm.tile([128, 4], f32, tag="slk")
                    nc.vector.tensor_scalar_mul(slk[:], slp[:], NEG)
                    nc.vector.tensor_tensor(out=slp[:], in0=slp[:], in1=slk[:],
                                            op=Alu.max)
                    pl = sm.tile([128, 4], f32, tag="pl")
                    nc.scalar.activation(out=pl[:], in_=slp[:], func=Act.Exp)
                    tmp132 = sm.tile([128, 132], f32, tag="tmp132")
                    nc.vector.tensor_tensor(
                        out=tmp132[:].rearrange("p (h c) -> p h c", c=33),
                        in0=TL[:, 0:132].rearrange("p (h c) -> p h c", c=33),
                        in1=pl[:].unsqueeze(2).to_broadcast([128, 4, 33]),
                        op=Alu.mult)
                    nc.vector.tensor_tensor(out=agg[:], in0=agg[:], in1=tmp132[:],
                                            op=Alu.add)
                    rec4 = sm.tile([128, 4], f32, tag="rec4")
                    nc.vector.reciprocal(
                        out=rec4[:],
                        in_=agg[:, 0:132].rearrange("p (h c) -> p h c", c=33)[:, :, 32])
                    xt = sm.tile([128, 128], f32, tag="xtm")
                    nc.vector.tensor_tensor(
                        out=xt[:].rearrange("p (h c) -> p h c", c=32),
                        in0=agg[:, 0:132].rearrange("p (h c) -> p h c", c=33)[:, :, 0:32],
                        in1=rec4[:].unsqueeze(2).to_broadcast([128, 4, 32]),
                        op=Alu.mult)
                    nc.vector.tensor_tensor(out=xt[:], in0=xt[:], in1=gbb[l][:],
                                            op=Alu.add)
                    xm = sm.tile([128, 128], f32, tag="xm")
                    nc.vector.tensor_scalar_min(xm[:], xt[:], 0.0)
                    xe = sm.tile([128, 128], f32, tag="xe")
                    nc.scalar.activation(out=xe[:], in_=xm[:], func=Act.Exp)
                    xr = sm.tile([128, 128], f32, tag="xr")
                    nc.vector.tensor_scalar_max(xr[:], xt[:], 0.0)
                    nc.vector.tensor_tensor(out=xr[:], in0=xr[:], in1=xe[:],
                                            op=Alu.add)
                    nc.vector.tensor_scalar_add(
                        xcur[:, g * 128:(g + 1) * 128], xr[:], -1.0)

                if l + 1 < L:
                    dense_phase(l + 1)

            # ---------------- heads -----------------------------------
            lg_sb = per.tile([128, TPC * 8], f32)
            gps = psg.tile([128, 1], f32, tag="gps")
            for t in range(TPC):
                xT_ps = psd.tile([128, 128], f32, tag="mm")
                nc.tensor.transpose(out=xT_ps[:], in_=xcur[:, t * 128:(t + 1) * 128],
                                    identity=ident[:])
                xT = sm.tile([128, 128], f32, tag="xT")
                nc.vector.tensor_copy(out=xT[:], in_=xT_ps[:])
                h1ps = psd.tile([128, 128], f32, tag="mm")
                nc.tensor.matmul(out=h1ps[:], lhsT=ahw1[:], rhs=xT[:],
                                 start=True, stop=True)
                h1T = sm.tile([128, 128], f32, tag="h1T")
                nc.scalar.activation(out=h1T[:], in_=h1ps[:], func=Act.Relu,
                                     bias=ahb1[:, 0:1])
                lgps = psd.tile([128, 8], f32, tag="mm")
                nc.tensor.matmul(out=lgps[:], lhsT=h1T[:], rhs=ahw2[:],
                                 start=True, stop=True)
                nc.vector.tensor_tensor(out=lg_sb[:, t * 8:(t + 1) * 8],
                                        in0=lgps[:], in1=ahb2b[:], op=Alu.add)
                nc.tensor.matmul(out=gps[:], lhsT=xcur[:, t * 128:(t + 1) * 128],
                                 rhs=nmask[:, t:t + 1],
                                 start=(t == 0), stop=(t == TPC - 1))
            nc.sync.dma_start(
                out=logit_d[:].rearrange("(t p) f -> p t f", p=128),
                in_=lg_sb[:].rearrange("p (t f) -> p t f", f=8))
            nc.sync.dma_start(
                out=embs_d[:].rearrange("(t p) f -> p t f", p=128),
                in_=xcur[:].rearrange("p (t f) -> p t f", f=128))

            gsb = sm.tile([128, 1], f32, tag="gsb")
            nc.vector.tensor_copy(out=gsb[:], in_=gps[:])
            nc.sync.dma_start(out=gsum_in[:], in_=gsb[:])
            nc.gpsimd.collective_compute("AllReduce", Alu.add, replica_groups=RG,
                                         ins=[gsum_in[:]], outs=[gsum_out[:]])
            gsT = sm.tile([128, 1], f32, tag="gsT")
            nc.sync.dma_start(out=gsT[:], in_=gsum_out[:])
            g1 = sm.tile([128, 1], f32, tag="g1")
            nc.vector.tensor_scalar_mul(g1[:], gsT[:], 1.0 / cfg.NREAL)
            vps = psd.tile([128, 1], f32, tag="mm")
            nc.tensor.matmul(out=vps[:], lhsT=vhw1[:], rhs=g1[:], start=True,
                             stop=True)
            g2 = sm.tile([128, 1], f32, tag="g2")
            nc.scalar.activation(out=g2[:], in_=vps[:], func=Act.Relu,
                                 bias=vhb1[:, 0:1])
            vps2 = psd.tile([1, 1], f32, tag="mm")
            nc.tensor.matmul(out=vps2[:], lhsT=vhw2[:], rhs=g2[:], start=True,
                             stop=True)
            val = sm.tile([1, 1], f32, tag="val")
            nc.vector.tensor_tensor(out=val[:], in0=vps2[:], in1=vhb2[:],
                                    op=Alu.add)
            nc.sync.dma_start(out=value_d[:], in_=val[:])

    nc.finalize()
    return nc


# ---------------------------------------------------------------- in_maps
def make_in_maps(cfg, inputs, cores):
    nf = np.asarray(inputs["node_features"], np.float32)
    nf_pad = np.zeros((cfg.NV, 3), np.float32)
    nf_pad[:cfg.NREAL] = nf
    nmask_full = np.zeros(cfg.NV, np.float32)
    nmask_full[:cfg.NREAL] = 1.0

    def f32c(x, shape=None):
        a = np.ascontiguousarray(np.asarray(x, np.float32))
        if shape is not None:
            a = a.reshape(shape)
        return a

    common = dict(
        lw3=f32c(inputs["gat_lin_w"], (3 * HID, HID)),
        lew3=f32c(inputs["gat_lin_edge_w"], (3 * HID, HID)),
        atts=f32c(inputs["gat_att_src"], (3 * H, C)),
        attd=f32c(inputs["gat_att_dst"], (3 * H, C)),
        atte=f32c(inputs["gat_att_edge"], (3 * H, C)),
        gbias=f32c(inputs["gat_bias"], (3, HID)),
        npw=f32c(inputs["np_w"]),
        npb=f32c(inputs["np_b"], (1, HID)),
        epw=f32c(inputs["ep_w"]),
        epb=f32c(inputs["ep_b"], (1, HID)),
        ahw1=f32c(inputs["ah_w1"]),
        ahb1=f32c(inputs["ah_b1"], (HID, 1)),
        ahw2=f32c(inputs["ah_w2"]),
        ahb2=f32c(inputs["ah_b2"], (1, 8)),
        vhw1=f32c(inputs["vh_w1"]),
        vhb1=f32c(inputs["vh_b1"], (HID, 1)),
        vhw2=f32c(inputs["vh_w2"]),
        vhb2=f32c(inputs["vh_b2"], (1, 1)),
    )
    in_maps = []
    for k in range(NCORES):
        cd = cores[k]
        nfk = nf_pad[k * cfg.NPC:(k + 1) * cfg.NPC]
        nmk = nmask_full[k * cfg.NPC:(k + 1) * cfg.NPC]
        m = dict(common)
        m["nf"] = np.ascontiguousarray(nfk)
        m["nmask"] = np.ascontiguousarray(
            nmk.reshape(cfg.TPC, 128).T.astype(np.float32))
        m["idxmain"] = cd["idxmain"]
        m["dloc"] = cd["dloc"]
        m["eav"] = cd["eav"]
        in_maps.append(m)
    return in_maps


def assemble_outputs(cfg, results):
    embs = np.concatenate([r["out_embs"] for r in results], 0)[:cfg.NREAL]
    logits = np.concatenate([r["out_logits"] for r in results], 0)[:cfg.NREAL]
    value = results[0]["out_value"]
    return logits.reshape(-1).astype(np.float32), value.astype(np.float32), \
        embs.astype(np.float32)


# ---------------------------------------------------------------- entry
def kernel(**inputs):
    cfg = CFG_FULL
    cores, slo, shi, B2 = host_prep(cfg, inputs["edge_index"],
                                    inputs["edge_attr"])
    nc = build_nc(cfg, slo, shi)
    in_maps = make_in_maps(cfg, inputs, cores)
    from concourse.bass_utils import run_bass_kernel_spmd
    res = run_bass_kernel_spmd(nc, in_maps, core_ids=list(range(NCORES)),
                               trace=bool(int(os.environ.get("KBENCH_TRACE", "0"))))
    out = assemble_outputs(cfg, res.results)
    if res.exec_time_ns is not None:
        print(f"HW exec time: {res.exec_time_ns} ns "
              f"(mean {res.mean_exec_time_ns} ns)")
    return out
